# revision 12
# baseline (speedup 1.0000x reference)
"""DAGNN (gnn_message_passing) Trainium2 kernel — 8 NeuronCores.

Per core (dst-shard of N/8 nodes):
  * MLP (PE, bf16): x-shard -> h [16, SH] feature-major.
  * K propagation hops:
      - AllGather h shards (f16) -> per-group gather tables
        [128p = 8 src-eighths x 16 feats, SH] (f32)
      - GpSimd ap_gather per slot-segment (edges grouped by src-eighth, laid
        out in degree-sorted windows; width template shared across all
        (core, group) pairs so one tensor_reduce covers all 128 partitions)
      - DVE: attr multiply + windowed tensor_reduce -> L1 partials (f32)
      - GpSimd ap_gather realign (degree-rank order -> global dst order)
      - DVE partition-tree add across the 8 groups -> h' [16, SH]
  * Combine: XBAR transpose-load of pps (f16), score = sigmoid(pps @ w + b),
    out = sum_k score_k * pps_k (f32).

h is rescaled by 1/16 per hop (folded into attr on host) to keep f16 pps in
range; the combine phase multiplies 16^k back in f32.
"""

import math
import numpy as np

import ml_dtypes

# problem sizes (overridable for scaled-down simulation tests)
N = 100000
E = 3200000
F_IN = 256
HID = 128
C = 16
K = 10
NCORES = 8
CHUNK_SLOTS = 2048          # max gather segment size
DEBUG_TAPS = False

bf16 = ml_dtypes.bfloat16
f16 = np.float16


def _dims(n):
    sh = n // NCORES
    shp = ((sh + 127) // 128) * 128
    rchunk = shp // 8 if (shp % 8 == 0 and (shp // 8) % 16 == 0) else shp
    return sh, shp, rchunk


# ----------------------------------------------------------------------------
# Host-side preprocessing
# ----------------------------------------------------------------------------

def _preprocess(edge_index, edge_attr):
    SH, SHP, RCHUNK = _dims(N)
    src = np.asarray(edge_index[0], dtype=np.int64)
    dst = np.asarray(edge_index[1], dtype=np.int64)
    attr = np.asarray(edge_attr, dtype=np.float64) / 16.0

    core_of = dst // SH
    grp_of = src // SH
    dloc = dst - core_of * SH
    sloc = src - grp_of * SH

    cg = (core_of * NCORES + grp_of).astype(np.int64)
    deg = np.zeros((NCORES * NCORES, SH), np.int64)
    np.add.at(deg, (cg, dloc), 1)
    deg = deg.reshape(NCORES, NCORES, SH)

    order = np.argsort(-deg, axis=2, kind="stable")          # [8,8,SH]
    rank = np.empty_like(order)
    ar = np.arange(SH)
    for c in range(NCORES):
        for g in range(NCORES):
            rank[c, g, order[c, g]] = ar
    sdeg = np.take_along_axis(deg, order, axis=2)
    W = np.maximum(sdeg.max(axis=(0, 1)), 1).astype(np.int64)  # non-increasing

    # segments: (w, n_windows, slot_off, l1_off); slot_off%16==0, n*w%16==0
    segs = []
    rank_l1 = np.zeros(SH, np.int64)
    slot_off = 0
    l1_off = 0
    bounds = [0] + list(np.flatnonzero(np.diff(W)) + 1) + [SH]
    for bi in range(len(bounds) - 1):
        i0, i1 = bounds[bi], bounds[bi + 1]
        w = int(W[i0])
        a = 32 // math.gcd(w, 32)
        per = max(a, (CHUNK_SLOTS // (a * w)) * a)
        assert per * w <= max(CHUNK_SLOTS, a * w)
        i = i0
        while i < i1:
            nn = min(per, i1 - i)
            n_pad = ((nn + a - 1) // a) * a
            rank_l1[i:i + nn] = l1_off + np.arange(nn)
            segs.append((w, n_pad, slot_off, l1_off))
            slot_off += n_pad * w
            l1_off += n_pad
            i += nn
    SLOTS = slot_off
    L1N = ((l1_off + 15) // 16) * 16
    assert SLOTS % 32 == 0 and L1N < 32768
    for (_w, _n, _so, _) in segs:
        assert _so % 32 == 0 and (_n * _w) % 32 == 0

    seg_w = np.zeros(l1_off, np.int64)
    seg_so = np.zeros(l1_off, np.int64)
    seg_lo = np.zeros(l1_off, np.int64)
    for (w, nn, so, lo) in segs:
        seg_w[lo:lo + nn] = w
        seg_so[lo:lo + nn] = so
        seg_lo[lo:lo + nn] = lo
    slotstart = seg_so[rank_l1] + (rank_l1 - seg_lo[rank_l1]) * seg_w[rank_l1]

    gidx = np.zeros((NCORES, NCORES, SLOTS), np.int16)
    arep = np.zeros((NCORES, NCORES, SLOTS), np.float32)
    e_sorted = np.argsort(cg, kind="stable")
    bnd = np.searchsorted(cg[e_sorted], np.arange(NCORES * NCORES + 1))
    for c in range(NCORES):
        for g in range(NCORES):
            kk = c * NCORES + g
            idx_e = e_sorted[bnd[kk]:bnd[kk + 1]]
            r = rank[c, g][dloc[idx_e]]
            o = np.argsort(r, kind="stable")
            idx_e, r = idx_e[o], r[o]
            startrun = np.flatnonzero(np.diff(r, prepend=-1))
            runlen = np.diff(np.append(startrun, len(r)))
            within = np.arange(len(r)) - np.repeat(startrun, runlen)
            pos = slotstart[r] + within
            gidx[c, g, pos] = sloc[idx_e].astype(np.int16)
            arep[c, g, pos] = attr[idx_e]

    RIDX_N = SHP
    ridx = np.zeros((NCORES, NCORES, RIDX_N), np.int16)
    for c in range(NCORES):
        for g in range(NCORES):
            ridx[c, g, :SH] = rank_l1[rank[c, g]].astype(np.int16)

    def wrap(a):  # [8, L] -> [128, L//16] in ap_gather "(s p)" layout
        L = a.shape[-1]
        return np.ascontiguousarray(
            a.reshape(NCORES, L // 16, 16).transpose(0, 2, 1).reshape(128, L // 16))

    gidx_w = np.stack([wrap(gidx[c]) for c in range(NCORES)])
    ridx_w = np.stack([wrap(ridx[c]) for c in range(NCORES)])
    arep_r = np.repeat(arep[:, :, None, :], C, axis=2).reshape(
        NCORES, 128, SLOTS).astype(f16)

    return dict(segs=segs, SLOTS=SLOTS, L1N=L1N,
                gidx=gidx_w, ridx=ridx_w, attr=arep_r)


# ----------------------------------------------------------------------------
# Bass program
# ----------------------------------------------------------------------------

def _build_program(segs, SLOTS, L1N):
    import concourse.bass as bass
    import concourse.bacc as bacc
    import concourse.tile as tile
    from concourse import mybir

    SH, SHP, RCHUNK = _dims(N)
    dt = mybir.dt
    AF = mybir.ActivationFunctionType
    ALU = mybir.AluOpType
    GLEN = max(nn * w for (w, nn, _, _) in segs)
    NSUBP = SHP // 128

    nc = bacc.Bacc("TRN2", target_bir_lowering=False, debug=False,
                   num_devices=NCORES)

    xsh = nc.declare_dram_parameter("xsh", [SHP, F_IN], dt.float32, isOutput=False)
    w1t = nc.declare_dram_parameter("w1t", [F_IN, HID], dt.bfloat16, isOutput=False)
    w2t = nc.declare_dram_parameter("w2t", [HID, HID], dt.bfloat16, isOutput=False)
    w3t = nc.declare_dram_parameter("w3t", [HID, C], dt.bfloat16, isOutput=False)
    b1 = nc.declare_dram_parameter("b1", [HID, 1], dt.float32, isOutput=False)
    b2 = nc.declare_dram_parameter("b2", [HID, 1], dt.float32, isOutput=False)
    b3 = nc.declare_dram_parameter("b3", [C, 1], dt.float32, isOutput=False)
    ident = nc.declare_dram_parameter("ident", [128, 128], dt.bfloat16, isOutput=False)
    gidx_d = nc.declare_dram_parameter("gidx", [128, SLOTS // 16], dt.int16, isOutput=False)
    ridx_d = nc.declare_dram_parameter("ridx", [128, SHP // 16], dt.int16, isOutput=False)
    attr_d = nc.declare_dram_parameter("attr", [128, SLOTS], dt.float16, isOutput=False)
    wk_d = nc.declare_dram_parameter("wk", [128, K + 1, C], dt.float32, isOutput=False)
    sc_d = nc.declare_dram_parameter("sc", [128, K + 1], dt.float32, isOutput=False)
    pb_d = nc.declare_dram_parameter("pb", [128, 1], dt.float32, isOutput=False)
    out_d = nc.declare_dram_parameter("out", [SHP, C], dt.float32, isOutput=True)
    if DEBUG_TAPS:
        dbg_d = nc.declare_dram_parameter("dbg", [K + 1, C, SHP], dt.float16,
                                          isOutput=True)
        dbg2_d = nc.declare_dram_parameter("dbg2", [NCORES, C, SH], dt.float16,
                                           isOutput=True)

    shard_d = nc.dram_tensor("shard_hbm", [C, SH], dt.float16)
    gath_d = nc.dram_tensor("gath_hbm", [NCORES, C, SH], dt.float16,
                            addr_space="Shared")
    pps_d = [nc.dram_tensor(f"pps{k}_hbm", [C, SHP], dt.float16)
             for k in range(K + 1)]

    groups = [list(range(NCORES))]

    with tile.TileContext(nc) as tc:
        with tc.tile_pool(name="const", bufs=1) as constp:
            gidx_sb = constp.tile([128, SLOTS // 16], dt.int16)
            nc.sync.dma_start(out=gidx_sb[:], in_=gidx_d[:])
            ridx_sb = constp.tile([128, SHP // 16], dt.int16)
            nc.sync.dma_start(out=ridx_sb[:], in_=ridx_d[:])

            # ---------- MLP ----------
            with (
                tc.tile_pool(name="mlp", bufs=2) as mlpp,
                tc.tile_pool(name="mlpc", bufs=1) as mlpc,
                tc.tile_pool(name="mpsum", bufs=2, space="PSUM") as mpsum,
            ):
                ident_sb = mlpc.tile([128, 128], dt.bfloat16)
                nc.sync.dma_start(out=ident_sb[:], in_=ident[:])
                w1_sb = mlpc.tile([128, 2, HID], dt.bfloat16)
                nc.sync.dma_start(out=w1_sb[:],
                                  in_=w1t.rearrange("(a p) m -> p a m", p=128))
                w2_sb = mlpc.tile([HID, HID], dt.bfloat16)
                nc.sync.dma_start(out=w2_sb[:], in_=w2t[:])
                w3_sb = mlpc.tile([HID, C], dt.bfloat16)
                nc.sync.dma_start(out=w3_sb[:], in_=w3t[:])
                b1_sb = mlpc.tile([HID, 1], dt.float32)
                nc.sync.dma_start(out=b1_sb[:], in_=b1[:])
                b2_sb = mlpc.tile([HID, 1], dt.float32)
                nc.sync.dma_start(out=b2_sb[:], in_=b2[:])
                b3_sb = mlpc.tile([C, 1], dt.float32)
                nc.sync.dma_start(out=b3_sb[:], in_=b3[:])
                hsb = mlpc.tile([C, SHP], dt.float16)

                coff = 0
                while coff < SHP:
                    cn = min(512, SHP - coff)
                    nsub = cn // 128
                    xin = mlpp.tile([128, nsub, F_IN], dt.float32, tag="xin")
                    nc.sync.dma_start(
                        out=xin[:],
                        in_=xsh.rearrange("(a p) f -> p a f", p=128)[
                            :, coff // 128:coff // 128 + nsub, :])
                    xbf = mlpp.tile([128, nsub, F_IN], dt.bfloat16, tag="xbf")
                    nc.vector.tensor_copy(xbf[:], xin[:])
                    xT = mlpp.tile([128, 2, cn], dt.bfloat16, tag="xT")
                    for s in range(nsub):
                        for hlf in range(2):
                            tp = mpsum.tile([128, 128], dt.bfloat16, tag="tp")
                            nc.tensor.transpose(
                                tp[:], xbf[:, s, hlf * 128:(hlf + 1) * 128],
                                ident_sb[:])
                            nc.scalar.activation(
                                xT[:, hlf, s * 128:(s + 1) * 128], tp[:], AF.Copy)
                    h1p = mpsum.tile([HID, cn], dt.float32, tag="h1p")
                    nc.tensor.matmul(h1p[:], w1_sb[:, 0, :], xT[:, 0, :],
                                     start=True, stop=False)
                    nc.tensor.matmul(h1p[:], w1_sb[:, 1, :], xT[:, 1, :],
                                     start=False, stop=True)
                    x2 = mlpp.tile([HID, cn], dt.bfloat16, tag="x2")
                    nc.scalar.activation(x2[:], h1p[:], AF.Relu, bias=b1_sb[:])
                    h2p = mpsum.tile([HID, cn], dt.float32, tag="h1p")
                    nc.tensor.matmul(h2p[:], w2_sb[:], x2[:], start=True, stop=True)
                    x3 = mlpp.tile([HID, cn], dt.bfloat16, tag="x2")
                    nc.scalar.activation(x3[:], h2p[:], AF.Relu, bias=b2_sb[:])
                    h3p = mpsum.tile([C, cn], dt.float32, tag="h3p")
                    nc.tensor.matmul(h3p[:], w3_sb[:], x3[:], start=True, stop=True)
                    nc.scalar.activation(hsb[:, coff:coff + cn], h3p[:], AF.Relu,
                                         bias=b3_sb[:])
                    coff += cn
                if SHP > SH:
                    nc.vector.memset(hsb[:, SH:SHP], 0.0)
                nc.sync.dma_start(out=pps_d[0][:], in_=hsb[:])
                nc.sync.dma_start(out=shard_d[:], in_=hsb[:, 0:SH])
                if DEBUG_TAPS:
                    nc.sync.dma_start(out=dbg_d[0], in_=hsb[:])

            # ---------- propagation hops ----------
            with (
                tc.tile_pool(name="pers", bufs=1) as pers,
                tc.tile_pool(name="hopp", bufs=2) as hopp,
                tc.tile_pool(name="alp", bufs=1) as alp,
            ):
                table = pers.tile([128, SH, 1], dt.float32)
                l1out = pers.tile([128, L1N, 1], dt.float32)
                ppsk = pers.tile([C, SHP], dt.float16)
                nc.vector.memset(l1out[:], 0.0)

                for k in range(1, K + 1):
                    nc.gpsimd.collective_compute(
                        "AllGather", ALU.bypass, replica_groups=groups,
                        ins=[shard_d.ap()], outs=[gath_d.ap()])
                    nc.gpsimd.dma_start(
                        out=table[:, :, 0],
                        in_=gath_d.rearrange("a b c -> (a b) c"))

                    for (w, nn, so, lo) in segs:
                        ln = nn * w
                        g = hopp.tile([128, GLEN, 1], dt.float32, tag="g")
                        nc.gpsimd.ap_gather(
                            g[:, 0:ln, :], table[:],
                            gidx_sb[:, so // 16:(so + ln) // 16],
                            channels=128, num_elems=SH, d=1, num_idxs=ln)
                        asb = hopp.tile([128, GLEN], dt.float16, tag="asb")
                        nc.scalar.dma_start(out=asb[:, 0:ln],
                                            in_=attr_d[:, so:so + ln])
                        nc.vector.tensor_mul(g[:, 0:ln, 0], g[:, 0:ln, 0],
                                             asb[:, 0:ln])
                        nc.vector.tensor_reduce(
                            l1out[:, lo:lo + nn, 0],
                            g[:, 0:ln, 0].rearrange("p (n w) -> p n w", w=w),
                            axis=mybir.AxisListType.X, op=ALU.add)

                    for rc in range(SHP // RCHUNK):
                        r0 = rc * RCHUNK
                        al = alp.tile([128, RCHUNK, 1], dt.float32, tag="al")
                        nc.gpsimd.ap_gather(
                            al[:], l1out[:],
                            ridx_sb[:, r0 // 16:(r0 + RCHUNK) // 16],
                            channels=128, num_elems=L1N, d=1, num_idxs=RCHUNK)
                        tmp64 = alp.tile([64, RCHUNK], dt.float32, tag="tmp64")
                        nc.sync.dma_start(out=tmp64[:], in_=al[64:128, :, 0])
                        t64 = alp.tile([64, RCHUNK], dt.float32, tag="t64")
                        nc.vector.tensor_add(t64[:], al[0:64, :, 0], tmp64[:])
                        tmp32 = alp.tile([32, RCHUNK], dt.float32, tag="tmp32")
                        nc.sync.dma_start(out=tmp32[:], in_=t64[32:64, :])
                        t32 = alp.tile([32, RCHUNK], dt.float32, tag="t32")
                        nc.vector.tensor_add(t32[:], t64[0:32, :], tmp32[:])
                        t16 = alp.tile([16, RCHUNK], dt.float32, tag="t16")
                        nc.sync.dma_start(out=t16[:], in_=t32[16:32, :])
                        nc.vector.tensor_add(
                            ppsk[:, r0:r0 + RCHUNK], t32[0:16, :], t16[:])
                    if SHP > SH:
                        nc.vector.memset(ppsk[:, SH:SHP], 0.0)
                    nc.sync.dma_start(out=pps_d[k][:], in_=ppsk[:])
                    if DEBUG_TAPS:
                        nc.sync.dma_start(out=dbg_d[k], in_=ppsk[:])
                        if k == 1:
                            nc.sync.dma_start(out=dbg2_d[:], in_=gath_d[:])
                    if k < K:
                        nc.sync.dma_start(out=shard_d[:], in_=ppsk[:, 0:SH])

        # ---------- combine ----------
        with (
            tc.tile_pool(name="comb", bufs=1) as comb,
            tc.tile_pool(name="combw", bufs=1) as combw,
        ):
            ppsT = comb.tile([128, NSUBP, K + 1, C], dt.float16)
            for k in range(K + 1):
                nc.sync.dma_start_transpose(ppsT[:, :, k, :], pps_d[k][:])
            wk_sb = comb.tile([128, K + 1, C], dt.float32)
            nc.sync.dma_start(out=wk_sb[:], in_=wk_d[:])
            sc_sb = comb.tile([128, K + 1], dt.float32)
            nc.sync.dma_start(out=sc_sb[:], in_=sc_d[:])
            pb_sb = comb.tile([128, 1], dt.float32)
            nc.sync.dma_start(out=pb_sb[:], in_=pb_d[:])

            shape4 = [128, NSUBP, K + 1, C]
            pps32 = comb.tile(shape4, dt.float32)
            nc.vector.tensor_mul(
                pps32[:], ppsT[:],
                sc_sb[:].unsqueeze(1).unsqueeze(3).broadcast_to(shape4))
            prod = combw.tile(shape4, dt.float32, tag="prod")
            nc.vector.tensor_mul(
                prod[:], pps32[:],
                wk_sb[:].unsqueeze(1).broadcast_to(shape4))
            spre = comb.tile([128, NSUBP, K + 1], dt.float32)
            nc.vector.tensor_reduce(spre[:], prod[:], axis=mybir.AxisListType.X,
                                    op=ALU.add)
            score = comb.tile([128, NSUBP, K + 1], dt.float32)
            nc.scalar.activation(score[:], spre[:], AF.Sigmoid, bias=pb_sb[:])
            prod2 = combw.tile(shape4, dt.float32, tag="prod")
            nc.vector.tensor_mul(
                prod2[:], pps32[:],
                score[:].unsqueeze(3).broadcast_to(shape4))
            outsb = comb.tile([128, NSUBP, C], dt.float32)
            nc.vector.tensor_reduce(outsb[:], prod2[:].transpose([0, 1, 3, 2]),
                                    axis=mybir.AxisListType.X, op=ALU.add)
            nc.sync.dma_start(
                out=out_d.rearrange("(a p) c -> p a c", p=128), in_=outsb[:])

    nc.compile()
    return nc


# ----------------------------------------------------------------------------
# Entry point
# ----------------------------------------------------------------------------

def _make_in_maps(pre, x, lin1_w, lin1_b, lin2_w, lin2_b, lin3_w, lin3_b,
                  proj_w, proj_b):
    SH, SHP, _ = _dims(N)
    x = np.asarray(x, np.float32)
    scale16 = (16.0 ** np.arange(K + 1)).astype(np.float32)
    wk = np.broadcast_to(
        np.asarray(proj_w, np.float32)[0][None, None, :], (128, K + 1, C)).copy()
    sc = np.broadcast_to(scale16[None, :], (128, K + 1)).copy()
    pb = np.full((128, 1), np.asarray(proj_b, np.float32)[0], np.float32)
    common = dict(
        w1t=np.ascontiguousarray(np.asarray(lin1_w, np.float32).T).astype(bf16),
        w2t=np.ascontiguousarray(np.asarray(lin2_w, np.float32).T).astype(bf16),
        w3t=np.ascontiguousarray(np.asarray(lin3_w, np.float32).T).astype(bf16),
        b1=np.asarray(lin1_b, np.float32).reshape(HID, 1),
        b2=np.asarray(lin2_b, np.float32).reshape(HID, 1),
        b3=np.asarray(lin3_b, np.float32).reshape(C, 1),
        ident=np.eye(128, dtype=bf16),
        wk=wk, sc=sc, pb=pb,
    )
    in_maps = []
    for c in range(NCORES):
        xp = np.zeros((SHP, F_IN), np.float32)
        xp[:SH] = x[c * SH:(c + 1) * SH]
        in_maps.append(dict(common, xsh=xp,
                            gidx=pre["gidx"][c], ridx=pre["ridx"][c],
                            attr=pre["attr"][c]))
    return in_maps


_CACHE = {}


def _run(trace=False, **inputs):
    from concourse.bass_utils import run_bass_kernel_spmd

    SH, _, _ = _dims(N)
    pre = _preprocess(inputs["edge_index"], inputs["edge_attr"])
    key = (pre["SLOTS"], pre["L1N"], tuple(pre["segs"]))
    if key not in _CACHE:
        _CACHE[key] = _build_program(pre["segs"], pre["SLOTS"], pre["L1N"])
    nc = _CACHE[key]

    in_maps = _make_in_maps(
        pre, inputs["x"], inputs["lin1_w"], inputs["lin1_b"],
        inputs["lin2_w"], inputs["lin2_b"], inputs["lin3_w"], inputs["lin3_b"],
        inputs["proj_w"], inputs["proj_b"])
    res = run_bass_kernel_spmd(nc, in_maps, list(range(NCORES)), trace=trace)
    out = np.concatenate([res.results[c]["out"][:SH] for c in range(NCORES)],
                         axis=0)
    return out.astype(np.float32), res


def kernel(x, edge_index, edge_attr, lin1_w, lin1_b, lin2_w, lin2_b,
           lin3_w, lin3_b, proj_w, proj_b):
    out, _ = _run(x=x, edge_index=edge_index, edge_attr=edge_attr,
                  lin1_w=lin1_w, lin1_b=lin1_b, lin2_w=lin2_w, lin2_b=lin2_b,
                  lin3_w=lin3_w, lin3_b=lin3_b, proj_w=proj_w, proj_b=proj_b)
    return out
